# revision 1
# baseline (speedup 1.0000x reference)
"""Trainium2 Bass kernel for nn_EnhancementLayerMamba (L=1 Mamba enhancement layer).

Strategy (8 NeuronCores, tensor-parallel over internal dim E=2048):

With sequence length 1 the selective scan collapses:
    y = delta * u * (Bm . Cm) + u * D        (A_log is dead: h0 = 0)

Host-side constant folding (weight-only transforms):
    Wi'   = diag(ln_g) @ W_in                 (fold LayerNorm gain)
    G     = W_x[:, :dt_rank] @ W_dt           (delta comes straight from u)
    Wod   = W_out @ W_d                       (mamba_out only feeds W_d)
    Wfbo  = W_f[:, N:] @ W_o                  (FiLM additive branch)
    hb_d  = b_out @ W_d + b_d ; hb_o = b_f[N:] @ W_o + b_o

Collectives (all mesh, 8 ranks):
    warmup AllReduce (128B)  - absorbs SPMD launch skew during the weight DMAs
    AllGather(u_k)  64KB/rank - full u everywhere; delta_k = softplus(u @ G_k)
    ReduceScatter([s_k; A_k; B_k]) - A = Wod^T(delta*u*gate) partials,
        B = Wod^T(u*D*gate) + W_d^T x partials, s_k = Bm_k . Cm_k over this
        core's 64-state slice.  s is a per-batch scalar so it commutes past
        the Wod contraction AND past the RS: z_pre = s * sumA + sumB.
    final output partials (contraction over N and COND sharded) summed on host.

All activations in [feature, batch=128] layout; every matmul is
lhsT = weight tile [K<=128, M<=128], rhs = activation [K, 128].
Weights arrive as a few pre-swizzled [128, C] packs (one DMA each).
"""

import json

import numpy as np
import ml_dtypes
from contextlib import ExitStack

import concourse.bass as bass
import concourse.mybir as mybir
import concourse.tile as tile
import concourse.bass_utils as _bass_utils
import concourse.bass2jax as _bass2jax
from concourse.bass_utils import run_bass_kernel_spmd

R = 8            # cores
B = 128          # batch (always the free dim)
STEPS = 1024
E = 2048
ES = E // R      # 256: E-shard per core (2 partition tiles j=0,1)
DTR = 512        # dt_rank
N = 512          # model states
ZS = N // R      # 64: z-shard per core
COND = 512

F32 = mybir.dt.float32
BF16 = mybir.dt.bfloat16
AF = mybir.ActivationFunctionType
ALU = mybir.AluOpType
GROUPS = [list(range(R))]

BF = ml_dtypes.bfloat16

# vec column map (f32, [128, 16]); cols 14/15 use partitions 0:64 only
V_SCL0, V_SCL1, V_BXC0, V_BXC1, V_BRS0, V_BRS1 = 0, 1, 2, 3, 4, 5
V_BDT0, V_BDT1, V_DD0, V_DD1 = 6, 7, 8, 9
V_CS = 10        # 10..13: colsum of Wi' column-tile m
V_BFG, V_HBD = 14, 15

# wp2 column offsets (bf16 cols)
O_G, O_WB, O_WC, O_WOD = 0, 4096, 5120, 6144
C2 = 7168
# wp3 column offsets
O_WD, O_WFG, O_CT, O_XS, O_FBO, O_WO, O_CTS = 0, 512, 768, 1280, 1408, 2432, 3456
C3 = 3584


def _split_multiwaits(bir_bytes: bytes) -> bytes:
    """The walrus in this image accepts one sync-wait per instruction
    ("Too many sync wait commands", CoreV3GenImpl setupSyncWait). Tile emits
    instructions with several waits; split the extras into single-wait
    EventSemaphore instructions on the same engine, directly before."""
    j = json.loads(bir_bytes)

    def fix(obj):
        if isinstance(obj, dict):
            for k, v in obj.items():
                if k == "instructions" and isinstance(v, list):
                    new = []
                    for ins in v:
                        si = ins.get("sync_info") if isinstance(ins, dict) else None
                        waits = si.get("on_wait") if si else None
                        if waits and len(waits) > 1:
                            for i, w in enumerate(waits[:-1]):
                                new.append({
                                    "debug": ins.get("debug", 0),
                                    "engine": ins["engine"],
                                    "ins": [], "outs": [],
                                    "name": f"{ins['name']}_w{i}",
                                    "opcode": "EventSemaphore",
                                    "sync_info": {"on_update": [],
                                                  "on_wait": [w]},
                                })
                            si["on_wait"] = waits[-1:]
                        new.append(ins)
                    obj[k] = new
                else:
                    fix(v)
        elif isinstance(obj, list):
            for v in obj:
                fix(v)

    fix(j)
    return json.dumps(j).encode()


_ORIG_COMPILE_BIR = _bass_utils.compile_bir_kernel


def _patched_compile_bir_kernel(bir_json, tmpdir, neff_name="file.neff"):
    if isinstance(bir_json, str):
        bir_json = _split_multiwaits(bir_json.encode())
    else:
        bir_json = _split_multiwaits(bytes(bir_json))
    return _ORIG_COMPILE_BIR(bir_json, tmpdir, neff_name=neff_name)


if getattr(_bass_utils.compile_bir_kernel, "__name__", "") != "_patched_compile_bir_kernel":
    _bass_utils.compile_bir_kernel = _patched_compile_bir_kernel
    _bass2jax.compile_bir_kernel = _patched_compile_bir_kernel


DEBUG = False


def build_nc() -> bass.Bass:
    nc = bass.Bass(num_devices=R)

    xp_d = nc.dram_tensor("xp", [128, STEPS], BF16, kind="ExternalInput")
    wp1_d = nc.dram_tensor("wp1", [128, 4096], BF16, kind="ExternalInput")
    wp2_d = nc.dram_tensor("wp2", [128, C2], BF16, kind="ExternalInput")
    wp3_d = nc.dram_tensor("wp3", [128, C3], BF16, kind="ExternalInput")
    vec_d = nc.dram_tensor("vec", [128, 16], F32, kind="ExternalInput")

    out_d = nc.dram_tensor("outp", [128, STEPS], BF16, kind="ExternalOutput")

    warm_d = nc.dram_tensor("warm", [8, 16], F32, kind="Internal")
    ag_in_d = nc.dram_tensor("ag_in", [2, 128, B], BF16, kind="Internal")
    ag_out_d = nc.dram_tensor("ag_out", [E, B], BF16, kind="Internal")
    rs_in_d = nc.dram_tensor("rs_in", [8, 129, B], BF16, kind="Internal")
    rs_out_d = nc.dram_tensor("rs_out", [129, B], BF16, kind="Internal")

    with ExitStack() as ctx:
        tc = ctx.enter_context(tile.TileContext(nc))
        wp = ctx.enter_context(tc.tile_pool(name="w", bufs=1))
        ap = ctx.enter_context(tc.tile_pool(name="a", bufs=1))
        pmm = ctx.enter_context(tc.tile_pool(name="pmm", bufs=4, space="PSUM"))
        pax = ctx.enter_context(tc.tile_pool(name="pax", bufs=2, space="PSUM"))
        pbc = ctx.enter_context(tc.tile_pool(name="pbc", bufs=2, space="PSUM"))

        # ---- constants (vector) ----
        ones_cb = wp.tile([128, 1], BF16, name="ones_cb", tag="ones_cb")
        nc.vector.memset(ones_cb[:], 1.0)
        ones_row = wp.tile([1, 128], F32, name="ones_row", tag="ones_row")
        nc.vector.memset(ones_row[:], 1.0)
        obf = wp.tile([1, 64], BF16, name="obf", tag="obf")
        nc.vector.memset(obf[:], 1.0)
        o648 = wp.tile([64, 8], F32, name="o648", tag="o648")
        nc.vector.memset(o648[:], 1.0)

        # ---- gpsimd: warmup collective to absorb SPMD launch skew ----
        w0 = wp.tile([8, 16], F32, name="w0", tag="w0")
        nc.gpsimd.memset(w0[:], 0.0)
        nc.gpsimd.dma_start(warm_d[:, :], w0[:])
        nc.gpsimd.collective_compute(
            "AllReduce", ALU.add, replica_groups=GROUPS,
            ins=[warm_d[:, :].opt()], outs=[warm_d[:, :].opt()])

        # ---- input DMAs: Wi halves split across both HWDGE rings so GEMM1
        # can start ~6us earlier; everything else follows ----
        xb = ap.tile([128, STEPS], BF16, name="xb", tag="xb")
        nc.sync.dma_start(xb[:], xp_d[:, :])
        wi = wp.tile([128, 4096], BF16, name="wi", tag="wi")
        nc.sync.dma_start(wi[:, :2048], wp1_d[:, :2048])
        vec = wp.tile([128, 16], F32, name="vec", tag="vec")
        nc.scalar.dma_start(vec[:], vec_d[:, :])
        nc.scalar.dma_start(wi[:, 2048:], wp1_d[:, 2048:])
        w2 = wp.tile([128, C2], BF16, name="w2", tag="w2")
        nc.scalar.dma_start(w2[:], wp2_d[:, :])
        w3 = wp.tile([128, C3], BF16, name="w3", tag="w3")
        nc.sync.dma_start(w3[:], wp3_d[:, :])

        # scalar: prefetch the ln/exp table set during the DMA wait
        junk = ap.tile([1, 128], F32, name="junk", tag="junk")
        nc.scalar.activation(junk[:], ones_row[:], AF.Ln, bias=1.0)

        # ---- LayerNorm stats (ones-matmul cross-partition reduce) ----
        sx_ps = pax.tile([1, B], F32, name="sx_ps", tag="pax")
        for k in range(8):
            nc.tensor.matmul(sx_ps[:], ones_cb[:], xb[:, B * k:B * (k + 1)],
                             start=(k == 0), stop=(k == 7))
        sq = [ap.tile([128, B], BF16, name=f"sq{k}", tag=f"sq{k}")
              for k in range(8)]
        for k in range(8):
            nc.vector.tensor_mul(sq[k][:], xb[:, B * k:B * (k + 1)],
                                 xb[:, B * k:B * (k + 1)])
        sx2_ps = pax.tile([1, B], F32, name="sx2_ps", tag="pax")
        for k in range(8):
            nc.tensor.matmul(sx2_ps[:], ones_cb[:], sq[k][:],
                             start=(k == 0), stop=(k == 7))

        mean = ap.tile([1, B], F32, name="mean", tag="mean")
        nc.vector.tensor_scalar_mul(mean[:], sx_ps[:], 1.0 / STEPS)
        ex2 = ap.tile([1, B], F32, name="ex2", tag="ex2")
        nc.vector.tensor_scalar_mul(ex2[:], sx2_ps[:], 1.0 / STEPS)
        m2 = ap.tile([1, B], F32, name="m2", tag="m2")
        nc.vector.tensor_mul(m2[:], mean[:], mean[:])
        var = ap.tile([1, B], F32, name="var", tag="var")
        nc.vector.tensor_sub(var[:], ex2[:], m2[:])
        vare = ap.tile([1, B], F32, name="vare", tag="vare")
        nc.vector.tensor_scalar_add(vare[:], var[:], 1e-5)
        # rsqrt = exp(-0.5 * ln(v)) - stays inside the ln/exp table set
        lnv = ap.tile([1, B], F32, name="lnv", tag="lnv")
        nc.scalar.activation(lnv[:], vare[:], AF.Ln)
        inv = ap.tile([1, B], F32, name="inv", tag="inv")
        nc.scalar.activation(inv[:], lnv[:], AF.Exp, scale=-0.5)
        mi = ap.tile([1, B], F32, name="mi", tag="mi")
        nc.vector.tensor_mul(mi[:], mean[:], inv[:])
        nmi = ap.tile([1, B], F32, name="nmi", tag="nmi")
        nc.vector.tensor_scalar_mul(nmi[:], mi[:], -1.0)

        invbc_ps = pbc.tile([128, B], F32, name="invbc_ps", tag="pbc")
        nc.tensor.matmul(invbc_ps[:], ones_row[:], inv[:], start=True, stop=True)
        invbc = ap.tile([128, B], F32, name="invbc", tag="invbc")
        nc.vector.tensor_copy(invbc[:], invbc_ps[:])
        nmibc_ps = pbc.tile([128, B], F32, name="nmibc_ps", tag="pbc")
        nc.tensor.matmul(nmibc_ps[:], ones_row[:], nmi[:], start=True, stop=True)
        nmibc = ap.tile([128, B], F32, name="nmibc", tag="nmibc")
        nc.vector.tensor_copy(nmibc[:], nmibc_ps[:])

        # scalar: prefetch the Silu table while GEMM1 runs (reads inv so the
        # scheduler cannot hoist it before the real Ln/Exp pair)
        nc.scalar.activation(junk[:], inv[:], AF.Silu)

        # ---- GEMM1: xz = x @ Wi\' (raw x, LN folded in post-scale) ----
        ub = ap.tile([128, 2 * B], BF16, name="ub", tag="ub")
        gate = []
        P_t, DP_t = [], []
        xz_sb = []
        for m in range(4):
            ps = pmm.tile([128, B], F32, name=f"xz{m}", tag="mm")
            for k in range(8):
                nc.tensor.matmul(
                    ps[:], wi[:, 512 * k + 128 * m:512 * k + 128 * (m + 1)],
                    xb[:, B * k:B * (k + 1)], start=(k == 0), stop=(k == 7))
            t1 = ap.tile([128, B], F32, name=f"xzt{m}", tag=f"xzt{m}")
            nc.vector.tensor_mul(t1[:], ps[:], invbc[:])
            xz = ap.tile([128, B], F32, name=f"xzs{m}", tag=f"xzs{m}")
            nc.vector.scalar_tensor_tensor(
                xz[:], nmibc[:], vec[:, V_CS + m:V_CS + m + 1], t1[:],
                ALU.mult, ALU.add)
            xz_sb.append(xz)
            if m < 2:
                # u = silu(conv_scale * xz + conv_bias)  (bf16, feeds AG + mms)
                nc.scalar.activation(ub[:, B * m:B * (m + 1)], xz[:], AF.Silu,
                                     bias=vec[:, V_BXC0 + m:V_BXC0 + m + 1],
                                     scale=vec[:, V_SCL0 + m:V_SCL0 + m + 1])

        # AG input bounce (per chunk, parallel HWDGE rings) + trigger
        nc.sync.dma_start(
            ag_in_d[0:1, :, :].rearrange("j p b -> p j b"),
            ub[:, 0:B].rearrange("p (j b) -> p j b", b=B))
        nc.scalar.dma_start(
            ag_in_d[1:2, :, :].rearrange("j p b -> p j b"),
            ub[:, B:2 * B].rearrange("p (j b) -> p j b", b=B))
        nc.gpsimd.collective_compute(
            "AllGather", ALU.bypass, replica_groups=GROUPS,
            ins=[ag_in_d[:, :, :].opt()], outs=[ag_out_d[:, :].opt()])

        # gate = silu(xz_res + b_res); P = u*gate; DP = P*D
        for j in range(2):
            gt = ap.tile([128, B], F32, name=f"gt{j}", tag=f"gt{j}")
            nc.scalar.activation(gt[:], xz_sb[2 + j][:], AF.Silu,
                                 bias=vec[:, V_BRS0 + j:V_BRS0 + j + 1])
            gate.append(gt)
        for j in range(2):
            pt = ap.tile([128, B], F32, name=f"pt{j}", tag=f"pt{j}")
            nc.vector.tensor_mul(pt[:], ub[:, B * j:B * (j + 1)], gate[j][:])
            dpt = ap.tile([128, B], BF16, name=f"dpt{j}", tag=f"dpt{j}")
            nc.vector.tensor_scalar_mul(dpt[:], pt[:],
                                        vec[:, V_DD0 + j:V_DD0 + j + 1])
            P_t.append(pt)
            DP_t.append(dpt)

        # ---- work inside the AG window ----
        # B_k = Wod^T (u*D*gate) + W_d^T x   (4 psum tiles of [128,B])
        B_sb = ap.tile([128, N], BF16, name="B_sb", tag="B_sb")
        for m in range(4):
            ps = pmm.tile([128, B], F32, name=f"bp{m}", tag="mm")
            nc.tensor.matmul(ps[:], w2[:, O_WOD + 128 * m:O_WOD + 128 * (m + 1)],
                             DP_t[0][:], start=True, stop=False)
            nc.tensor.matmul(ps[:], w2[:, O_WOD + 512 + 128 * m:O_WOD + 512 + 128 * (m + 1)],
                             DP_t[1][:], start=False, stop=False)
            nc.tensor.matmul(ps[:], w3[:, O_WD + 128 * m:O_WD + 128 * (m + 1)],
                             w3[:, O_XS:O_XS + B], start=False, stop=True)
            dst = B_sb[:, 128 * m:128 * (m + 1)]
            if m % 2 == 0:
                nc.vector.tensor_copy(dst, ps[:])
            else:
                nc.scalar.copy(dst, ps[:])
        # stage B into the RS buffer early (pays its DMA cost in the AG gap)
        for h in range(2):
            eng = nc.scalar if h == 0 else nc.sync
            eng.dma_start(
                rs_in_d[4 * h:4 * h + 4, 65:129, :].rearrange("j r b -> r j b"),
                B_sb[64 * h:64 * h + 64, :].rearrange("p (m c) -> p m c", c=B))

        # FiLM additive branch: fbo = Wfbo^T cts  (COND-sharded partial)
        fbo_sb = ap.tile([128, STEPS], BF16, name="fbo_sb", tag="fbo_sb")
        for m in range(8):
            fps = pmm.tile([128, B], F32, name=f"fb{m}", tag="mm")
            nc.tensor.matmul(fps[:], w3[0:64, O_FBO + 128 * m:O_FBO + 128 * (m + 1)],
                             w3[0:64, O_CTS:O_CTS + B], start=True, stop=True)
            dst = fbo_sb[:, 128 * m:128 * (m + 1)]
            if m % 2 == 0:
                nc.vector.tensor_copy(dst, fps[:])
            else:
                nc.scalar.copy(dst, fps[:])

        # FiLM gain: gg = Wfg^T c + b_fg   (this core\'s 64 states)
        g_ps = pax.tile([64, B], F32, name="g_ps", tag="pax")
        for k in range(4):
            nc.tensor.matmul(g_ps[:], w3[:, O_WFG + 64 * k:O_WFG + 64 * (k + 1)],
                             w3[:, O_CT + B * k:O_CT + B * (k + 1)],
                             start=(k == 0), stop=(k == 3))
        gg = ap.tile([64, B], BF16, name="gg", tag="gg")
        nc.vector.tensor_scalar_add(gg[:], g_ps[:], vec[0:64, V_BFG:V_BFG + 1])

        # scalar: prefetch the exp+ln table set (softplus) in the AG window;
        # reading gate[1] pins it after the real silu calls
        nc.scalar.activation(junk[:], gate[1][0:1, :], AF.Exp)

        # ---- AG return: full u; Bm/Cm first (s-chain hides under delta) ----
        uf = ap.tile([128, 16 * B], BF16, name="uf", tag="uf")
        for q in range(4):
            eng = nc.sync if q % 2 == 0 else nc.scalar
            eng.dma_start(
                uf[:, 4 * B * q:4 * B * (q + 1)].rearrange("p (c b) -> p c b", b=B),
                ag_out_d[512 * q:512 * (q + 1), :].rearrange("(c p) b -> p c b", p=128))

        bm_ps = pax.tile([64, B], F32, name="bm_ps", tag="pax")
        for kk in range(16):
            nc.tensor.matmul(bm_ps[:], w2[:, O_WB + 64 * kk:O_WB + 64 * (kk + 1)],
                             uf[:, B * kk:B * (kk + 1)],
                             start=(kk == 0), stop=(kk == 15))
        cm_ps = pax.tile([64, B], F32, name="cm_ps", tag="pax")
        for kk in range(16):
            nc.tensor.matmul(cm_ps[:], w2[:, O_WC + 64 * kk:O_WC + 64 * (kk + 1)],
                             uf[:, B * kk:B * (kk + 1)],
                             start=(kk == 0), stop=(kk == 15))
        cm_sb = ap.tile([64, B], F32, name="cm_sb", tag="cm_sb")
        nc.vector.tensor_copy(cm_sb[:], cm_ps[:])
        bcm = ap.tile([64, B], F32, name="bcm", tag="bcm")
        nc.vector.tensor_mul(bcm[:], bm_ps[:], cm_sb[:])
        s8_ps = pax.tile([8, B], F32, name="s8_ps", tag="pax")
        nc.tensor.matmul(s8_ps[:], o648[:], bcm[:], start=True, stop=True)
        s8_sb = ap.tile([8, B], BF16, name="s8_sb", tag="s8_sb")
        nc.vector.tensor_copy(s8_sb[:], s8_ps[:])
        nc.gpsimd.dma_start(rs_in_d[:, 0:1, :],
                            s8_sb[:].rearrange("p (o b) -> p o b", o=1))

        # delta_j = softplus(u_full @ G[:, j-tile] + b_dt)
        q_t = []
        for m in range(2):
            ps = pmm.tile([128, B], F32, name=f"dl{m}", tag="mm")
            for kk in range(16):
                nc.tensor.matmul(ps[:], w2[:, O_G + 256 * kk + 128 * m:O_G + 256 * kk + 128 * (m + 1)],
                                 uf[:, B * kk:B * (kk + 1)],
                                 start=(kk == 0), stop=(kk == 15))
            ex = ap.tile([128, B], F32, name=f"dex{m}", tag=f"dex{m}")
            nc.scalar.activation(ex[:], ps[:], AF.Exp,
                                 bias=vec[:, V_BDT0 + m:V_BDT0 + m + 1])
            dl = ap.tile([128, B], F32, name=f"dlt{m}", tag=f"dlt{m}")
            nc.scalar.activation(dl[:], ex[:], AF.Ln, bias=1.0)
            qb = ap.tile([128, B], BF16, name=f"qb{m}", tag=f"qb{m}")
            nc.vector.tensor_mul(qb[:], dl[:], P_t[m][:])
            q_t.append(qb)

        # A_k = Wod^T (delta*u*gate)
        A_sb = ap.tile([128, N], BF16, name="A_sb", tag="A_sb")
        for m in range(4):
            ps = pmm.tile([128, B], F32, name=f"ap{m}", tag="mm")
            nc.tensor.matmul(ps[:], w2[:, O_WOD + 128 * m:O_WOD + 128 * (m + 1)],
                             q_t[0][:], start=True, stop=False)
            nc.tensor.matmul(ps[:], w2[:, O_WOD + 512 + 128 * m:O_WOD + 512 + 128 * (m + 1)],
                             q_t[1][:], start=False, stop=True)
            if m % 2 == 0:
                nc.vector.tensor_copy(A_sb[:, 128 * m:128 * (m + 1)], ps[:])
            else:
                nc.scalar.copy(A_sb[:, 128 * m:128 * (m + 1)], ps[:])

        # scalar: prefetch Gelu table (reads A_sb so its load runs during RS)
        nc.scalar.activation(junk[:], A_sb[0:1, 0:B], AF.Gelu)

        # ---- ReduceScatter of [s; A; B] blocks ----
        for h in range(2):
            eng = nc.sync if h == 0 else nc.scalar
            eng.dma_start(
                rs_in_d[4 * h:4 * h + 4, 1:65, :].rearrange("j r b -> r j b"),
                A_sb[64 * h:64 * h + 64, :].rearrange("p (m c) -> p m c", c=B))
        nc.gpsimd.collective_compute(
            "ReduceScatter", ALU.add, replica_groups=GROUPS,
            ins=[rs_in_d[:, :, :].opt()], outs=[rs_out_d[:, :].opt()])

        ztA = ap.tile([64, B], BF16, name="ztA", tag="ztA")
        nc.sync.dma_start(ztA[:], rs_out_d[1:65, :])
        ztB = ap.tile([64, B], BF16, name="ztB", tag="ztB")
        nc.scalar.dma_start(ztB[:], rs_out_d[65:129, :])
        s_t = ap.tile([1, B], BF16, name="s_t", tag="s_t")
        nc.gpsimd.dma_start(s_t[:], rs_out_d[0:1, :])

        # ---- tail: z = gelu(s*A + B + hb_d); FiLM; out partial ----
        sbc_ps = pax.tile([64, B], F32, name="sbc_ps", tag="pax")
        nc.tensor.matmul(sbc_ps[:], obf[:], s_t[:], start=True, stop=True)
        zp1 = ap.tile([64, B], F32, name="zp1", tag="zp1")
        nc.vector.tensor_mul(zp1[:], ztA[:], sbc_ps[:])
        zpre = ap.tile([64, B], F32, name="zpre", tag="zpre")
        nc.vector.tensor_add(zpre[:], zp1[:], ztB[:])
        z = ap.tile([64, B], F32, name="z", tag="z")
        nc.scalar.activation(z[:], zpre[:], AF.Gelu,
                             bias=vec[0:64, V_HBD:V_HBD + 1])
        zg = ap.tile([64, B], BF16, name="zg", tag="zg")
        nc.vector.tensor_mul(zg[:], z[:], gg[:])

        out_sb = ap.tile([128, STEPS], BF16, name="out_sb", tag="out_sb")
        for m in range(8):
            ps = pmm.tile([128, B], F32, name=f"o{m}", tag="mm")
            nc.tensor.matmul(ps[:], w3[0:64, O_WO + 128 * m:O_WO + 128 * (m + 1)],
                             zg[:], start=True, stop=True)
            nc.vector.tensor_add(out_sb[:, 128 * m:128 * (m + 1)], ps[:],
                                 fbo_sb[:, 128 * m:128 * (m + 1)])
        nc.sync.dma_start(out_d[:, :512], out_sb[:, :512])
        nc.scalar.dma_start(out_d[:, 512:], out_sb[:, 512:])

        if DEBUG:
            dbg = {
                "d_ub": (ub, [128, 2 * B], BF16),
                "d_uf": (uf, [128, 16 * B], BF16),
                "d_A": (A_sb, [128, N], BF16),
                "d_B": (B_sb, [128, N], BF16),
                "d_s8": (s8_sb, [8, B], BF16),
                "d_ztA": (ztA, [64, B], BF16),
                "d_ztB": (ztB, [64, B], BF16),
                "d_st": (s_t, [1, B], BF16),
                "d_gg": (gg, [64, B], BF16),
                "d_zg": (zg, [64, B], BF16),
            }
            for nm, (t, shp, dt) in dbg.items():
                dd = nc.dram_tensor(nm, shp, dt, kind="ExternalOutput")
                nc.sync.dma_start(dd[:, :], t[:])

    return nc


_CACHE = {}


def _get_nc() -> bass.Bass:
    if "nc" not in _CACHE:
        _CACHE["nc"] = build_nc()
    return _CACHE["nc"]


def _topack(w):
    """[r*128, C] row-major -> [128, r*C] SBUF pack (chunk-major columns)."""
    r = w.shape[0] // 128
    return np.ascontiguousarray(
        w.reshape(r, 128, -1).transpose(1, 0, 2).reshape(128, -1))


def kernel(**inputs) -> np.ndarray:
    inp = {k: np.asarray(v) for k, v in inputs.items()}
    f32 = np.float32
    x = inp["x"].reshape(B, STEPS).astype(f32)
    c = inp["c"].astype(f32)
    ln_g = inp["ln_g"].astype(f32)
    ln_b = inp["ln_b"].astype(f32)
    W_in = inp["W_in"].astype(f32)
    conv_w = inp["conv_w"].astype(f32)
    conv_b = inp["conv_b"].astype(f32)
    W_x = inp["W_x"].astype(f32)
    W_dt = inp["W_dt"].astype(f32)
    b_dt = inp["b_dt"].astype(f32)
    D = inp["D"].astype(f32)
    W_out = inp["W_out"].astype(f32)
    b_out = inp["b_out"].astype(f32)
    W_d = inp["W_d"].astype(f32)
    b_d = inp["b_d"].astype(f32)
    W_f = inp["W_f"].astype(f32)
    b_f = inp["b_f"].astype(f32)
    W_o = inp["W_o"].astype(f32)
    b_o = inp["b_o"].astype(f32)

    # host constant folding (weight-only)
    Wi_full = ln_g[:, None] * W_in          # (1024, 4096)
    bias_xz = ln_b @ W_in                   # (4096,)
    G = W_x[:, :DTR] @ W_dt                 # (2048, 2048)
    W_B = W_x[:, DTR:DTR + N]               # (2048, 512)
    W_C = W_x[:, DTR + N:]                  # (2048, 512)
    Wod = W_out @ W_d                       # (2048, 512)
    hb_d = b_out @ W_d + b_d                # (512,)
    W_fg = W_f[:, :N]
    b_fg = b_f[:N]
    W_fbo = W_f[:, N:] @ W_o                # (512, 1024)
    hb_o = b_f[N:] @ W_o + b_o              # (1024,)
    cw3 = conv_w[3, 0, :]                   # (2048,)

    xT = np.ascontiguousarray(x.T)          # (1024, 128)
    cT = np.ascontiguousarray(c.T)          # (512, 128)
    xp = _topack(xT).astype(BF)

    in_maps = []
    for k in range(R):
        es = slice(ES * k, ES * (k + 1))
        nz0 = 128 * (k % 4) + 64 * (k // 4)     # permuted z-shard start
        nz = slice(nz0, nz0 + ZS)
        cz = slice(ZS * k, ZS * (k + 1))        # COND shard (fbo branch)
        xr = slice(128 * k, 128 * (k + 1))

        Wi_k = np.concatenate([Wi_full[:, es], Wi_full[:, E:][:, es]], axis=1)
        bxz_xi = bias_xz[:E][es]
        bxz_res = bias_xz[E:][es]
        scl_k = cw3[es]
        bxc_k = bxz_xi * scl_k + conv_b[es]
        colsum_k = Wi_k.sum(axis=0)             # (512,)

        vec = np.zeros((128, 16), f32)
        for j in range(2):
            sl = slice(128 * j, 128 * (j + 1))
            vec[:, V_SCL0 + j] = scl_k[sl]
            vec[:, V_BXC0 + j] = bxc_k[sl]
            vec[:, V_BRS0 + j] = bxz_res[sl]
            vec[:, V_BDT0 + j] = b_dt[es][sl]
            vec[:, V_DD0 + j] = D[es][sl]
        for m in range(4):
            vec[:, V_CS + m] = colsum_k[128 * m:128 * (m + 1)]
        vec[0:64, V_BFG] = b_fg[nz]
        vec[0:64, V_HBD] = hb_d[nz]

        wp2 = np.zeros((128, C2), f32)
        wp2[:, O_G:O_G + 4096] = _topack(G[:, es])
        wp2[:, O_WB:O_WB + 1024] = _topack(W_B[:, nz])
        wp2[:, O_WC:O_WC + 1024] = _topack(W_C[:, nz])
        wp2[:, O_WOD:O_WOD + 1024] = _topack(Wod[es, :])

        wp3 = np.zeros((128, C3), f32)
        wp3[:, O_WD:O_WD + 512] = W_d[xr, :]
        wp3[:, O_WFG:O_WFG + 256] = _topack(W_fg[:, nz])
        wp3[:, O_CT:O_CT + 512] = _topack(cT)
        wp3[:, O_XS:O_XS + B] = xp[:, 128 * k:128 * (k + 1)]
        wp3[0:64, O_FBO:O_FBO + 1024] = W_fbo[cz, :]
        wp3[0:64, O_WO:O_WO + 1024] = W_o[nz, :]
        wp3[0:64, O_CTS:O_CTS + B] = cT[cz, :]

        in_maps.append({
            "xp": xp,
            "wp1": _topack(Wi_k).astype(BF),
            "wp2": wp2.astype(BF),
            "wp3": wp3.astype(BF),
            "vec": vec,
        })

    nc = _get_nc()
    res = run_bass_kernel_spmd(nc, in_maps, core_ids=list(range(R)),
                               **_CACHE.get("run_kwargs", {}))
    _CACHE["last_results"] = res
    out_T = np.zeros((STEPS, B), np.float64)
    for r in res.results:
        o = np.asarray(r["outp"], dtype=np.float64)        # [128, 1024]
        out_T += o.reshape(128, 8, 128).transpose(1, 0, 2).reshape(STEPS, B)
    out = out_T.T.astype(f32) + hb_o[None, :]
    return out.astype(f32)



# revision 5
# speedup vs baseline: 1.3329x; 1.3329x over previous
"""Trainium2 Bass kernel for nn_EnhancementLayerMamba (L=1 Mamba enhancement layer).

Strategy (8 NeuronCores): ZERO collectives. Profiling the tensor-parallel
variant showed its time dominated by cross-core costs (launch-skew barrier
~40us + 3 collectives at 10-12us each), while per-core local work finished
in <30us. So: replicate all (constant-folded, bf16) weights on every core
and shard the BATCH (16 rows per core). Each core's NTFF span is then just
its own DMA-bound weight stream (~21.5 MB @ ~400 GB/s) with the matmuls
(~775, weight-load-bound at free-dim 16, FWL active) hidden underneath.

With sequence length 1 the selective scan collapses:
    y = delta * u * (Bm . Cm) + u * D        (A_log is dead: h0 = 0)

Host-side constant folding (weight-only transforms):
    Wi'   = diag(ln_g) @ W_in, conv scale cw3 folded into the xi columns
    Wod   = W_out @ W_d                       (mamba_out only feeds W_d)
    Wfbo  = W_f[:, N:] @ W_o                  (FiLM additive branch)
    hb_d  = b_out @ W_d + b_d ; hb_o = b_f[N:] @ W_o + b_o
    (W_x / W_dt kept factored: 8 MB instead of a 16 MB dense G)

On-device per core (all activations [feature, batch=16], weights arrive
as m-major [128, 128]-tile packs so compute chases the DMA stream):
    LN stats -> h = (x-mean)*rsqrt(var+eps)   (gain/bias folded into Wi')
    xz = h @ Wi' ; u = silu(xz_xi + bias_xi) ; gate = silu(xz_res + b_res)
    P = u*gate ; DP = P*D
    [d_r; Bm; Cm] = u @ W_x ; s = Bm . Cm (per-batch scalar)
    delta = softplus(d_r @ W_dt + b_dt) ; q = delta*P
    A = Wod^T q ; B = Wod^T DP + W_d^T x ; z = gelu(s*A + B + hb_d)
    out = (z * (W_fg^T c + b_fg)) @ W_o + c @ Wfbo   (+ hb_o on host)
"""

import json

import numpy as np
import ml_dtypes
from contextlib import ExitStack

import concourse.bass as bass
import concourse.mybir as mybir
import concourse.tile as tile
import concourse.bass_utils as _bass_utils
import concourse.bass2jax as _bass2jax
from concourse.bass_utils import run_bass_kernel_spmd

R = 8            # cores
B = 128          # full batch
BL = 16          # per-core batch shard
STEPS = 1024
E = 2048
DTR = 512        # dt_rank
N = 512          # model states
COND = 512

KI = 8           # h/x k-chunks   (1024/128)
MI = 32          # GEMM1 m-groups (4096/128)
KU = 16          # u k-chunks     (2048/128)
MX = 12          # W_x m-groups   (1536/128)
MD = 16          # W_dt m-groups  (2048/128)
MA = 4           # N m-groups     (512/128)
KC = 4           # c k-chunks     (512/128)
MO = 8           # out m-groups   (1024/128)

F32 = mybir.dt.float32
BF16 = mybir.dt.bfloat16
AF = mybir.ActivationFunctionType
ALU = mybir.AluOpType

BF = ml_dtypes.bfloat16

# vec column map (f32, [128, 56])
V_BXI = 0        # 0..15  bias_xi per GEMM1 xi m-group
V_BRS = 16       # 16..31 bias_res per res m-group
V_BDT = 32       # 32..47 b_dt per delta m-group
V_HBD = 48       # 48..51 hb_d per N m-group
V_BFG = 52       # 52..55 b_fg per N m-group
NVEC = 56


def _split_multiwaits(bir_bytes: bytes) -> bytes:
    """The walrus in this image accepts one sync-wait per instruction
    ("Too many sync wait commands", CoreV3GenImpl setupSyncWait). Tile emits
    instructions with several waits; split the extras into single-wait
    EventSemaphore instructions on the same engine, directly before."""
    j = json.loads(bir_bytes)

    def fix(obj):
        if isinstance(obj, dict):
            for k, v in obj.items():
                if k == "instructions" and isinstance(v, list):
                    new = []
                    for ins in v:
                        si = ins.get("sync_info") if isinstance(ins, dict) else None
                        waits = si.get("on_wait") if si else None
                        if waits and len(waits) > 1:
                            for i, w in enumerate(waits[:-1]):
                                new.append({
                                    "debug": ins.get("debug", 0),
                                    "engine": ins["engine"],
                                    "ins": [], "outs": [],
                                    "name": f"{ins['name']}_w{i}",
                                    "opcode": "EventSemaphore",
                                    "sync_info": {"on_update": [],
                                                  "on_wait": [w]},
                                })
                            si["on_wait"] = waits[-1:]
                        new.append(ins)
                    obj[k] = new
                else:
                    fix(v)
        elif isinstance(obj, list):
            for v in obj:
                fix(v)

    fix(j)
    return json.dumps(j).encode()


_ORIG_COMPILE_BIR = _bass_utils.compile_bir_kernel


def _patched_compile_bir_kernel(bir_json, tmpdir, neff_name="file.neff"):
    if isinstance(bir_json, str):
        bir_json = _split_multiwaits(bir_json.encode())
    else:
        bir_json = _split_multiwaits(bytes(bir_json))
    return _ORIG_COMPILE_BIR(bir_json, tmpdir, neff_name=neff_name)


if getattr(_bass_utils.compile_bir_kernel, "__name__", "") != "_patched_compile_bir_kernel":
    _bass_utils.compile_bir_kernel = _patched_compile_bir_kernel
    _bass2jax.compile_bir_kernel = _patched_compile_bir_kernel


def build_nc() -> bass.Bass:
    nc = bass.Bass(num_devices=R)

    xp_d = nc.dram_tensor("xp", [128, KI * BL], BF16, kind="ExternalInput")
    cp_d = nc.dram_tensor("cp", [128, KC * BL], BF16, kind="ExternalInput")
    vec_d = nc.dram_tensor("vec", [128, NVEC], F32, kind="ExternalInput")
    dexp_d = nc.dram_tensor("dexp", [128, KU * BL], BF16, kind="ExternalInput")
    wi_d = nc.dram_tensor("wi", [128, MI * KI * 128], BF16, kind="ExternalInput")
    wx_d = nc.dram_tensor("wx", [128, MX * KU * 128], BF16, kind="ExternalInput")
    wdt_d = nc.dram_tensor("wdt", [128, MD * 4 * 128], BF16, kind="ExternalInput")
    wod_d = nc.dram_tensor("wod", [128, MA * KU * 128], BF16, kind="ExternalInput")
    wd_d = nc.dram_tensor("wd", [128, MA * KI * 128], BF16, kind="ExternalInput")
    wfg_d = nc.dram_tensor("wfg", [128, MA * KC * 128], BF16, kind="ExternalInput")
    wfbo_d = nc.dram_tensor("wfbo", [128, MO * KC * 128], BF16, kind="ExternalInput")
    wo_d = nc.dram_tensor("wo", [128, MO * KC * 128], BF16, kind="ExternalInput")

    out_d = nc.dram_tensor("outp", [128, MO * BL], BF16, kind="ExternalOutput")

    with ExitStack() as ctx:
        tc = ctx.enter_context(tile.TileContext(nc))
        wp = ctx.enter_context(tc.tile_pool(name="w", bufs=1))
        ap = ctx.enter_context(tc.tile_pool(name="a", bufs=1))
        pmm = ctx.enter_context(tc.tile_pool(name="pmm", bufs=4, space="PSUM"))
        pw = ctx.enter_context(tc.tile_pool(name="pw", bufs=1, space="PSUM"))
        pax = ctx.enter_context(tc.tile_pool(name="pax", bufs=1, space="PSUM"))
        pbc = ctx.enter_context(tc.tile_pool(name="pbc", bufs=2, space="PSUM"))

        # ---- constants ----
        ones_cb = wp.tile([128, 1], BF16, name="ones_cb", tag="ones_cb")
        nc.vector.memset(ones_cb[:], 1.0)
        ones_row = wp.tile([1, 128], F32, name="ones_row", tag="ones_row")
        nc.vector.memset(ones_row[:], 1.0)
        junk = ap.tile([1, BL], F32, name="junk", tag="junk")

        # ---- DMAs: two HWDGE rings (sync / scalar), FIFO per ring.
        # Small activation inputs first, then weights in consumption order,
        # column chunks alternating between the rings.
        xb = wp.tile([128, KI * BL], BF16, name="xb", tag="xb")
        nc.sync.dma_start(xb[:], xp_d[:, :])
        cbt = wp.tile([128, KC * BL], BF16, name="cbt", tag="cbt")
        nc.scalar.dma_start(cbt[:], cp_d[:, :])
        vec = wp.tile([128, NVEC], F32, name="vec", tag="vec")
        nc.sync.dma_start(vec[:], vec_d[:, :])
        dexp = wp.tile([128, KU * BL], BF16, name="dexp", tag="dexp")
        nc.scalar.dma_start(dexp[:], dexp_d[:, :])

        wi = wp.tile([128, MI * KI * 128], BF16, name="wi", tag="wi")
        for c in range(8):      # 4096-col chunks = 4 m-groups each
            eng = nc.sync if c % 2 == 0 else nc.scalar
            eng.dma_start(wi[:, 4096 * c:4096 * (c + 1)],
                          wi_d[:, 4096 * c:4096 * (c + 1)])
        wx = wp.tile([128, MX * KU * 128], BF16, name="wx", tag="wx")
        for c in range(6):      # 4096-col chunks = 2 m-groups each
            eng = nc.sync if c % 2 == 0 else nc.scalar
            eng.dma_start(wx[:, 4096 * c:4096 * (c + 1)],
                          wx_d[:, 4096 * c:4096 * (c + 1)])
        wdt = wp.tile([128, MD * 4 * 128], BF16, name="wdt", tag="wdt")
        nc.sync.dma_start(wdt[:, :4096], wdt_d[:, :4096])
        nc.scalar.dma_start(wdt[:, 4096:], wdt_d[:, 4096:])
        wod = wp.tile([128, MA * KU * 128], BF16, name="wod", tag="wod")
        nc.sync.dma_start(wod[:, :4096], wod_d[:, :4096])
        nc.scalar.dma_start(wod[:, 4096:], wod_d[:, 4096:])
        wd = wp.tile([128, MA * KI * 128], BF16, name="wd", tag="wd")
        nc.sync.dma_start(wd[:], wd_d[:, :])
        wfg = wp.tile([128, MA * KC * 128], BF16, name="wfg", tag="wfg")
        nc.scalar.dma_start(wfg[:], wfg_d[:, :])
        wfbo = wp.tile([128, MO * KC * 128], BF16, name="wfbo", tag="wfbo")
        nc.sync.dma_start(wfbo[:], wfbo_d[:, :])
        wo = wp.tile([128, MO * KC * 128], BF16, name="wo", tag="wo")
        nc.scalar.dma_start(wo[:], wo_d[:, :])

        # scalar: prefetch the ln/exp table set while DMAs stream
        nc.scalar.activation(junk[:], ones_row[0:1, 0:BL], AF.Ln, bias=1.0)

        # ---- LayerNorm stats (ones-matmul cross-partition reduce) ----
        sx_ps = pax.tile([1, BL], F32, name="sx_ps", tag="pax")
        for k in range(KI):
            nc.tensor.matmul(sx_ps[:], ones_cb[:], xb[:, BL * k:BL * (k + 1)],
                             start=(k == 0), stop=(k == KI - 1))
        sq = [ap.tile([128, BL], BF16, name=f"sq{k}", tag=f"sq{k}")
              for k in range(KI)]
        for k in range(KI):
            nc.vector.tensor_mul(sq[k][:], xb[:, BL * k:BL * (k + 1)],
                                 xb[:, BL * k:BL * (k + 1)])
        sx2_ps = pax.tile([1, BL], F32, name="sx2_ps", tag="pax")
        for k in range(KI):
            nc.tensor.matmul(sx2_ps[:], ones_cb[:], sq[k][:],
                             start=(k == 0), stop=(k == KI - 1))

        mean = ap.tile([1, BL], F32, name="mean", tag="mean")
        nc.vector.tensor_scalar_mul(mean[:], sx_ps[:], 1.0 / STEPS)
        ex2 = ap.tile([1, BL], F32, name="ex2", tag="ex2")
        nc.vector.tensor_scalar_mul(ex2[:], sx2_ps[:], 1.0 / STEPS)
        m2 = ap.tile([1, BL], F32, name="m2", tag="m2")
        nc.vector.tensor_mul(m2[:], mean[:], mean[:])
        var = ap.tile([1, BL], F32, name="var", tag="var")
        nc.vector.tensor_sub(var[:], ex2[:], m2[:])
        vare = ap.tile([1, BL], F32, name="vare", tag="vare")
        nc.vector.tensor_scalar_add(vare[:], var[:], 1e-5)
        # rsqrt = exp(-0.5 * ln(v)) - stays inside the ln/exp table set
        lnv = ap.tile([1, BL], F32, name="lnv", tag="lnv")
        nc.scalar.activation(lnv[:], vare[:], AF.Ln)
        inv = ap.tile([1, BL], F32, name="inv", tag="inv")
        nc.scalar.activation(inv[:], lnv[:], AF.Exp, scale=-0.5)

        meanbc_ps = pbc.tile([128, BL], F32, name="meanbc_ps", tag="pbc")
        nc.tensor.matmul(meanbc_ps[:], ones_row[:], mean[:], start=True, stop=True)
        meanbc = ap.tile([128, BL], F32, name="meanbc", tag="meanbc")
        nc.vector.tensor_copy(meanbc[:], meanbc_ps[:])
        invbc_ps = pbc.tile([128, BL], F32, name="invbc_ps", tag="pbc")
        nc.tensor.matmul(invbc_ps[:], ones_row[:], inv[:], start=True, stop=True)
        invbc = ap.tile([128, BL], F32, name="invbc", tag="invbc")
        nc.vector.tensor_copy(invbc[:], invbc_ps[:])

        h = ap.tile([128, KI * BL], BF16, name="h", tag="h")
        for k in range(KI):
            hd = ap.tile([128, BL], F32, name=f"hd{k}", tag=f"hd{k}")
            nc.vector.tensor_sub(hd[:], xb[:, BL * k:BL * (k + 1)], meanbc[:])
            nc.vector.tensor_mul(h[:, BL * k:BL * (k + 1)], hd[:], invbc[:])

        # scalar: prefetch the Silu table (reads inv so it runs after stats)
        nc.scalar.activation(junk[:], inv[:], AF.Silu)

        # ---- GEMM1: xz = h @ Wi' ; u / gate via silu with folded biases ----
        up = ap.tile([128, KU * BL], BF16, name="up", tag="up")
        gp = ap.tile([128, KU * BL], BF16, name="gp", tag="gp")
        for m in range(MI):
            ps = pmm.tile([128, BL], F32, name=f"xz{m}", tag="mm")
            for k in range(KI):
                nc.tensor.matmul(
                    ps[:], wi[:, 1024 * m + 128 * k:1024 * m + 128 * (k + 1)],
                    h[:, BL * k:BL * (k + 1)], start=(k == 0), stop=(k == KI - 1))
            if m < 16:
                nc.scalar.activation(up[:, BL * m:BL * (m + 1)], ps[:], AF.Silu,
                                     bias=vec[:, V_BXI + m:V_BXI + m + 1])
            else:
                j = m - 16
                nc.scalar.activation(gp[:, BL * j:BL * (j + 1)], ps[:], AF.Silu,
                                     bias=vec[:, V_BRS + j:V_BRS + j + 1])

        Pp = ap.tile([128, KU * BL], BF16, name="Pp", tag="Pp")
        nc.vector.tensor_mul(Pp[:], up[:], gp[:])
        DPp = ap.tile([128, KU * BL], BF16, name="DPp", tag="DPp")
        nc.vector.tensor_mul(DPp[:], Pp[:], dexp[:])

        # ---- W_x: [d_r; Bm; Cm] = u @ W_x ----
        drp = ap.tile([128, 4 * BL], BF16, name="drp", tag="drp")
        bc_ps = pw.tile([128, 8 * BL], F32, name="bc_ps", tag="pw")
        for m in range(MX):
            if m < 4:
                ps = pmm.tile([128, BL], F32, name=f"xd{m}", tag="mm")[:]
            else:
                ps = bc_ps[:, BL * (m - 4):BL * (m - 3)]
            for k in range(KU):
                nc.tensor.matmul(
                    ps, wx[:, 2048 * m + 128 * k:2048 * m + 128 * (k + 1)],
                    up[:, BL * k:BL * (k + 1)], start=(k == 0), stop=(k == KU - 1))
            if m < 4:
                nc.vector.tensor_copy(drp[:, BL * m:BL * (m + 1)], ps)

        # DVE may read only one PSUM operand: drain Bm to SBUF, mul with Cm
        bmsb = ap.tile([128, 4 * BL], F32, name="bmsb", tag="bmsb")
        for j in range(4):
            nc.vector.tensor_copy(bmsb[:, BL * j:BL * (j + 1)],
                                  bc_ps[:, BL * j:BL * (j + 1)])
        bcm = ap.tile([128, 4 * BL], BF16, name="bcm", tag="bcm")
        for j in range(4):
            nc.vector.tensor_mul(bcm[:, BL * j:BL * (j + 1)],
                                 bmsb[:, BL * j:BL * (j + 1)],
                                 bc_ps[:, BL * (4 + j):BL * (5 + j)])
        s_ps = pax.tile([1, BL], F32, name="s_ps", tag="pax")
        for j in range(4):
            nc.tensor.matmul(s_ps[:], ones_cb[:], bcm[:, BL * j:BL * (j + 1)],
                             start=(j == 0), stop=(j == 3))
        s_sb = ap.tile([1, BL], F32, name="s_sb", tag="s_sb")
        nc.vector.tensor_copy(s_sb[:], s_ps[:])
        sbc_ps = pbc.tile([128, BL], F32, name="sbc_ps", tag="pbc")
        nc.tensor.matmul(sbc_ps[:], ones_row[:], s_sb[:], start=True, stop=True)
        sbc = ap.tile([128, BL], F32, name="sbc", tag="sbc")
        nc.vector.tensor_copy(sbc[:], sbc_ps[:])

        # ---- W_dt: delta = softplus(d_r @ W_dt + b_dt) ; q = delta * P ----
        dpre = ap.tile([128, MD * BL], F32, name="dpre", tag="dpre")
        for m in range(MD):
            ps = pmm.tile([128, BL], F32, name=f"dt{m}", tag="mm")
            for k in range(4):
                nc.tensor.matmul(
                    ps[:], wdt[:, 512 * m + 128 * k:512 * m + 128 * (k + 1)],
                    drp[:, BL * k:BL * (k + 1)], start=(k == 0), stop=(k == 3))
            nc.vector.tensor_scalar_add(dpre[:, BL * m:BL * (m + 1)], ps[:],
                                        vec[:, V_BDT + m:V_BDT + m + 1])

        # scalar: swap back to the exp/ln table set (pinned after last silu)
        nc.scalar.activation(junk[:], gp[0:1, BL * 15:BL * 16], AF.Exp)

        expp = ap.tile([128, MD * BL], F32, name="expp", tag="expp")
        nc.scalar.activation(expp[:], dpre[:], AF.Exp)
        deltap = ap.tile([128, MD * BL], F32, name="deltap", tag="deltap")
        nc.scalar.activation(deltap[:], expp[:], AF.Ln, bias=1.0)
        qp = ap.tile([128, KU * BL], BF16, name="qp", tag="qp")
        nc.vector.tensor_mul(qp[:], deltap[:], Pp[:])

        # scalar: prefetch Gelu table (pinned after softplus ln)
        nc.scalar.activation(junk[:], deltap[0:1, 0:BL], AF.Gelu)

        # ---- A = Wod^T q ; B = Wod^T DP + W_d^T x ; z = gelu(s*A+B+hb_d) ----
        z = ap.tile([128, MA * BL], BF16, name="z", tag="z")
        for m in range(MA):
            psA = pmm.tile([128, BL], F32, name=f"A{m}", tag="mm")
            for k in range(KU):
                nc.tensor.matmul(
                    psA[:], wod[:, 2048 * m + 128 * k:2048 * m + 128 * (k + 1)],
                    qp[:, BL * k:BL * (k + 1)], start=(k == 0), stop=(k == KU - 1))
            psB = pmm.tile([128, BL], F32, name=f"B{m}", tag="mm")
            for k in range(KU):
                nc.tensor.matmul(
                    psB[:], wod[:, 2048 * m + 128 * k:2048 * m + 128 * (k + 1)],
                    DPp[:, BL * k:BL * (k + 1)], start=(k == 0), stop=False)
            for k in range(KI):
                nc.tensor.matmul(
                    psB[:], wd[:, 1024 * m + 128 * k:1024 * m + 128 * (k + 1)],
                    xb[:, BL * k:BL * (k + 1)], start=False, stop=(k == KI - 1))
            zp = ap.tile([128, BL], F32, name=f"zp{m}", tag=f"zp{m}")
            nc.vector.tensor_mul(zp[:], psA[:], sbc[:])
            zpre = ap.tile([128, BL], F32, name=f"zpre{m}", tag=f"zpre{m}")
            nc.vector.tensor_add(zpre[:], zp[:], psB[:])
            nc.scalar.activation(z[:, BL * m:BL * (m + 1)], zpre[:], AF.Gelu,
                                 bias=vec[:, V_HBD + m:V_HBD + m + 1])

        # ---- FiLM gain: gg = W_fg^T c + b_fg ; zg = z * gg ----
        gg = ap.tile([128, MA * BL], BF16, name="gg", tag="gg")
        for m in range(MA):
            ps = pmm.tile([128, BL], F32, name=f"g{m}", tag="mm")
            for k in range(KC):
                nc.tensor.matmul(
                    ps[:], wfg[:, 512 * m + 128 * k:512 * m + 128 * (k + 1)],
                    cbt[:, BL * k:BL * (k + 1)], start=(k == 0), stop=(k == KC - 1))
            nc.vector.tensor_scalar_add(gg[:, BL * m:BL * (m + 1)], ps[:],
                                        vec[:, V_BFG + m:V_BFG + m + 1])
        zgp = ap.tile([128, MA * BL], BF16, name="zgp", tag="zgp")
        nc.vector.tensor_mul(zgp[:], z[:], gg[:])

        # ---- out = W_o^T zg + Wfbo^T c ----
        out_sb = ap.tile([128, MO * BL], BF16, name="out_sb", tag="out_sb")
        for m in range(MO):
            ps = pmm.tile([128, BL], F32, name=f"o{m}", tag="mm")
            for k in range(KC):
                nc.tensor.matmul(
                    ps[:], wo[:, 512 * m + 128 * k:512 * m + 128 * (k + 1)],
                    zgp[:, BL * k:BL * (k + 1)], start=(k == 0), stop=False)
            for k in range(KC):
                nc.tensor.matmul(
                    ps[:], wfbo[:, 512 * m + 128 * k:512 * m + 128 * (k + 1)],
                    cbt[:, BL * k:BL * (k + 1)], start=False, stop=(k == KC - 1))
            dst = out_sb[:, BL * m:BL * (m + 1)]
            if m % 2 == 0:
                nc.vector.tensor_copy(dst, ps[:])
            else:
                nc.scalar.copy(dst, ps[:])
        nc.sync.dma_start(out_d[:, :MO * BL // 2], out_sb[:, :MO * BL // 2])
        nc.scalar.dma_start(out_d[:, MO * BL // 2:], out_sb[:, MO * BL // 2:])

    return nc


_CACHE = {}


def _get_nc() -> bass.Bass:
    if "nc" not in _CACHE:
        _CACHE["nc"] = build_nc()
    return _CACHE["nc"]


def _tiles(W, KT, MT):
    """[KT*128, MT*128] -> [128, MT*KT*128] m-major tile pack:
    tile (m,k) at cols (m*KT+k)*128, element [p,c] = W[k*128+p, m*128+c]."""
    return np.ascontiguousarray(
        W.reshape(KT, 128, MT, 128).transpose(1, 2, 0, 3)
        .reshape(128, MT * KT * 128))


def _kpack(T, KT):
    """[KT*128, n] -> [128, KT*n]: chunk k at cols k*n, [p,c]=T[k*128+p, c]."""
    n = T.shape[1]
    return np.ascontiguousarray(
        T.reshape(KT, 128, n).transpose(1, 0, 2).reshape(128, KT * n))


def kernel(**inputs) -> np.ndarray:
    inp = {k: np.asarray(v) for k, v in inputs.items()}
    f32 = np.float32
    x = inp["x"].reshape(B, STEPS).astype(f32)
    c = inp["c"].astype(f32)
    ln_g = inp["ln_g"].astype(f32)
    ln_b = inp["ln_b"].astype(f32)
    W_in = inp["W_in"].astype(f32)
    conv_w = inp["conv_w"].astype(f32)
    conv_b = inp["conv_b"].astype(f32)
    W_x = inp["W_x"].astype(f32)
    W_dt = inp["W_dt"].astype(f32)
    b_dt = inp["b_dt"].astype(f32)
    D = inp["D"].astype(f32)
    W_out = inp["W_out"].astype(f32)
    b_out = inp["b_out"].astype(f32)
    W_d = inp["W_d"].astype(f32)
    b_d = inp["b_d"].astype(f32)
    W_f = inp["W_f"].astype(f32)
    b_f = inp["b_f"].astype(f32)
    W_o = inp["W_o"].astype(f32)
    b_o = inp["b_o"].astype(f32)

    # host constant folding (weight-only)
    cw3 = conv_w[3, 0, :]                       # (2048,) last conv tap (L=1)
    Wi = ln_g[:, None] * W_in                   # (1024, 4096)
    Wi[:, :E] *= cw3[None, :]
    bias_xz = ln_b @ W_in                       # (4096,)
    bias_xi = bias_xz[:E] * cw3 + conv_b
    bias_res = bias_xz[E:]
    Wod = W_out @ W_d                           # (2048, 512)
    hb_d = b_out @ W_d + b_d                    # (512,)
    W_fg, b_fg = W_f[:, :N], b_f[:N]
    W_fbo = W_f[:, N:] @ W_o                    # (512, 1024)
    hb_o = b_f[N:] @ W_o + b_o                  # (1024,)

    vec = np.zeros((128, NVEC), f32)
    for m in range(16):
        vec[:, V_BXI + m] = bias_xi[128 * m:128 * (m + 1)]
        vec[:, V_BRS + m] = bias_res[128 * m:128 * (m + 1)]
        vec[:, V_BDT + m] = b_dt[128 * m:128 * (m + 1)]
    for m in range(MA):
        vec[:, V_HBD + m] = hb_d[128 * m:128 * (m + 1)]
        vec[:, V_BFG + m] = b_fg[128 * m:128 * (m + 1)]

    dexp = np.repeat(
        D.reshape(KU, 128).T[:, :, None], BL, axis=2).reshape(128, KU * BL)

    wpacks = {
        "vec": vec,
        "dexp": dexp.astype(BF),
        "wi": _tiles(Wi, KI, MI).astype(BF),
        "wx": _tiles(W_x, KU, MX).astype(BF),
        "wdt": _tiles(W_dt, 4, MD).astype(BF),
        "wod": _tiles(Wod, KU, MA).astype(BF),
        "wd": _tiles(W_d, KI, MA).astype(BF),
        "wfg": _tiles(W_fg, KC, MA).astype(BF),
        "wfbo": _tiles(W_fbo, KC, MO).astype(BF),
        "wo": _tiles(W_o, KC, MO).astype(BF),
    }

    in_maps = []
    for r in range(R):
        bs = slice(BL * r, BL * (r + 1))
        xT = np.ascontiguousarray(x[bs].T)      # (1024, 16)
        cT = np.ascontiguousarray(c[bs].T)      # (512, 16)
        in_maps.append({
            "xp": _kpack(xT, KI).astype(BF),
            "cp": _kpack(cT, KC).astype(BF),
            **wpacks,
        })

    nc = _get_nc()
    res = run_bass_kernel_spmd(nc, in_maps, core_ids=list(range(R)),
                               **_CACHE.get("run_kwargs", {}))
    _CACHE["last_results"] = res
    out = np.zeros((B, STEPS), f32)
    for r, rr in enumerate(res.results):
        o = np.asarray(rr["outp"], dtype=f32)   # [128, MO*BL]
        out[BL * r:BL * (r + 1)] = (
            o.reshape(128, MO, BL).transpose(1, 0, 2).reshape(STEPS, BL).T)
    return (out + hb_o[None, :]).astype(f32)
